# revision 6
# baseline (speedup 1.0000x reference)
"""DenseGeneralAqt inference kernel for Trainium2 (8 NeuronCores).

out = (x @ dequant_int8(qkernel)) * qscale,  x:(2,2048,1024) f32,
qkernel:(1024,4096) int8, qscale:(1,4096) f32 -> out:(2,2048,4096) f32.

Strategy: 2D sharding — 4-way over the flattened token axis (M) x 2-way
over features (N); per core a [1024,1024]x[1024,2048] fp16 GEMM whose
PE-streaming floor (256 matmuls of 512 cycles) dominates, so everything
else is arranged to hide under it. Host marshalling pre-packs all DRAM
operands in on-chip consumption order so every DMA is a contiguous
block: xT pair-major fp16 [128kp, pair, kt, 256m], weights kt-major
int8 [128kp, kt, 2048n], output tile-major f32 [32, 128, 512]. The PE
is heated with N=512 dummy matmuls (full duty cycle -> the HAM clock
gate releases 1.2->2.4 GHz right as the first inputs land); weight
k-tiles are dequantized int8->fp16 on the vector engine just ahead of
consumption; the fp16 per-channel scale is fused into the PSUM->SBUF
drain. Sweeps run k-outer across all 8 PSUM banks; the final sweep runs
n-outer and its last chain is split into two 256-wide halves so the
closing drain+store is short. Output stores alternate between the
Scalar and Sync HWDGE rings.
"""

import numpy as np

P = 128
B, S, D, F = 2, 2048, 1024, 4096
N_CORES = 8
MSH, NSH = 4, 2                   # shard grid: 4 m-blocks x 2 n-blocks
M_FULL = B * S                    # 4096 rows
M_CORE = M_FULL // MSH            # 1024 rows per core
N_CORE = F // NSH                 # 2048 cols per core
NT = 512                          # n-tile (one PSUM bank of f32)
WM, WK, WN = M_CORE // P, D // P, N_CORE // NT   # 8, 8, 4
NPAIR = WM // 2                   # 4 m-pair sweeps
MP = M_CORE // NPAIR              # 256 m per pair
NWARM = 10

_CACHE: dict = {}


def _build():
    import concourse.tile as tile
    from concourse import bacc, mybir

    nc = bacc.Bacc("TRN2", target_bir_lowering=False, debug=False)

    xt_dram = nc.dram_tensor("xt", [P, NPAIR, WK, MP], mybir.dt.float16, kind="ExternalInput")
    w_dram = nc.dram_tensor("w", [P, WK, N_CORE], mybir.dt.int8, kind="ExternalInput")
    s_dram = nc.dram_tensor("s", [1, N_CORE], mybir.dt.float16, kind="ExternalInput")
    o_dram = nc.dram_tensor("o", [WM * WN, P, NT], mybir.dt.float32, kind="ExternalOutput")

    with tile.TileContext(nc) as tc:
        with (
            tc.tile_pool(name="wi", bufs=1) as wip,
            tc.tile_pool(name="w", bufs=1) as wp,
            tc.tile_pool(name="qs", bufs=1) as qp,
            tc.tile_pool(name="xh", bufs=1) as xhp,
            tc.tile_pool(name="o", bufs=10) as op,
            tc.tile_pool(name="ps", bufs=8, space="PSUM") as pp,
        ):
            # Weights (int8, kt-major contiguous, 2KB-per-partition runs):
            # one DMA per k-tile, first thing on the GpSimd ring so k0
            # lands earliest.
            w_i8 = wip.tile([P, WK, N_CORE], mybir.dt.int8, name="wi", tag="wi")
            wd = [
                nc.gpsimd.dma_start(w_i8[:, kt, :], w_dram[:, kt, :])
                for kt in range(WK)
            ]

            # PE warm-up: N=512 dummy matmuls run the array at full duty
            # (LDWEIGHTS hides under streaming), so the HAM clock gate
            # reliably releases ~3.4us after the first one issues — right
            # when the first real inputs land. Memset on the vector
            # engine, whose first real work (dequant) is DMA-gated anyway.
            warm = wp.tile([P, NT], mybir.dt.float16, name="warm", tag="warm")
            nc.vector.memset(warm[:], 0)
            warm_ps = pp.tile([P, NT], mybir.dt.float32, name="warm_ps", tag="ps")
            for _ in range(NWARM):
                nc.tensor.matmul(warm_ps[:], warm[:, 0:P], warm[:])

            # xT shard, pair-major, in half-pair chunks (2KB-per-partition
            # runs). Interleave with the weight k-tiles in consumption
            # order via cross-engine issue deps so the shared DMA queues
            # deliver bytes in the order the PE consumes them.
            xh = xhp.tile([P, NPAIR, WK, MP], mybir.dt.float16, name="xh", tag="xh")
            xd = []
            for pr in range(NPAIR):
                for hk in range(2):
                    xd.append(nc.sync.dma_start(
                        xh[:, pr, 4 * hk:4 * hk + 4, :],
                        xt_dram[:, pr, 4 * hk:4 * hk + 4, :],
                    ))
            tile.add_dep_helper(xd[0].ins, wd[1].ins, reason="p0a after k1")
            tile.add_dep_helper(wd[2].ins, xd[0].ins, reason="k2 after p0a")
            tile.add_dep_helper(xd[1].ins, wd[3].ins, reason="p0b after k3")
            tile.add_dep_helper(wd[4].ins, xd[1].ins, reason="k4 after p0b")
            tile.add_dep_helper(xd[2].ins, wd[7].ins, reason="p1a after k7")

            # Dequant int8 -> fp16 on the vector engine, in k order.
            w_sb = wp.tile([P, WK, N_CORE], mybir.dt.float16, name="w", tag="w")
            cv = [nc.vector.tensor_copy(w_sb[:, kt, :], w_i8[:, kt, :]) for kt in range(WK)]

            # Per-channel scale, fp16, replicated across partitions by a
            # DRE-broadcast DMA; deferred so its bytes don't starve the
            # critical early loads, lands long before the first drain.
            qs = qp.tile([P, N_CORE], mybir.dt.float16)
            qs_dma = nc.scalar.dma_start(qs[:], s_dram[0:1, :].to_broadcast((P, N_CORE)))
            tile.add_dep_helper(qs_dma.ins, cv[2].ins, reason="defer qs broadcast")

            st_eng = [nc.scalar, nc.sync]

            def drain(pi, mh, nt, ps_ap, cols):
                mi = pi * 2 + mh
                j = mi * WN + nt
                ot = op.tile([P, NT], mybir.dt.float32, name=f"o{j}_{cols.start}", tag="o")
                nc.vector.tensor_mul(
                    ot[:, cols], ps_ap, qs[:, nt * NT + cols.start:nt * NT + cols.stop]
                )
                st_eng[(j + cols.start // NT) % 2].dma_start(
                    o_dram[j, :, cols], ot[:, cols]
                )

            def mm(ps_ap, pi, kt, mh, nslice, first, last):
                nc.tensor.matmul(
                    ps_ap,
                    xh[:, pi, kt, mh * P:(mh + 1) * P],
                    w_sb[:, kt, nslice],
                    start=first,
                    stop=last,
                )

            full = slice(0, NT)
            combos = [(mh, nt) for mh in (0, 1) for nt in range(WN)]
            for pi in range(NPAIR):
                if pi < NPAIR - 1:
                    # k-outer: consume each weight k-tile across all 8
                    # PSUM banks as soon as it is dequantized.
                    ps = {
                        c: pp.tile([P, NT], mybir.dt.float32, name=f"ps{pi}_{c[0]}_{c[1]}", tag="ps")
                        for c in combos
                    }
                    for kt in range(WK):
                        for (mh, nt) in combos:
                            mm(ps[(mh, nt)][:], pi, kt, mh,
                               slice(nt * NT, (nt + 1) * NT), kt == 0, kt == WK - 1)
                    for (mh, nt) in combos:
                        drain(pi, mh, nt, ps[(mh, nt)][:], full)
                else:
                    # Last sweep: n-outer so each bank finishes early and
                    # drains/stores overlap the remaining matmuls; the
                    # final chain is split into two 256-wide halves so
                    # the closing drain+store is half-length.
                    for (mh, nt) in combos[:-1]:
                        ps_t = pp.tile([P, NT], mybir.dt.float32, name=f"ps{pi}_{mh}_{nt}", tag="ps")
                        for kt in range(WK):
                            mm(ps_t[:], pi, kt, mh,
                               slice(nt * NT, (nt + 1) * NT), kt == 0, kt == WK - 1)
                        drain(pi, mh, nt, ps_t[:], full)
                    mh, nt = combos[-1]
                    for half in range(2):
                        cols = slice(half * (NT // 2), (half + 1) * (NT // 2))
                        ps_t = pp.tile([P, NT], mybir.dt.float32, name=f"ps{pi}_{mh}_{nt}_{half}", tag="ps")
                        for kt in range(WK):
                            mm(ps_t[:, cols], pi, kt, mh,
                               slice(nt * NT + cols.start, nt * NT + cols.stop),
                               kt == 0, kt == WK - 1)
                        drain(pi, mh, nt, ps_t[:, cols], cols)

    nc.compile()
    return nc


def _get_nc():
    if "nc" not in _CACHE:
        _CACHE["nc"] = _build()
    return _CACHE["nc"]


def _run(x, qkernel, qscale, trace=False):
    from concourse.bass_utils import run_bass_kernel_spmd

    x = np.asarray(x, dtype=np.float32).reshape(M_FULL, D).astype(np.float16)
    w = np.asarray(qkernel)
    if w.dtype != np.int8:
        w = w.astype(np.int8)
    s = np.asarray(qscale, dtype=np.float32).reshape(1, F).astype(np.float16)

    in_maps = []
    for c in range(N_CORES):
        mb, nb = c % MSH, c // MSH
        xm = x[mb * M_CORE:(mb + 1) * M_CORE]                  # [1024, 1024]
        # [kp, pair, kt, m']  <-  xm[pr*256+m', kt*128+kp]
        xt = np.ascontiguousarray(
            xm.reshape(NPAIR, MP, WK, P).transpose(3, 0, 2, 1)
        )
        wn = w[:, nb * N_CORE:(nb + 1) * N_CORE]               # [1024, 2048]
        wk = np.ascontiguousarray(wn.reshape(WK, P, N_CORE).transpose(1, 0, 2))
        in_maps.append({
            "xt": xt,
            "w": wk,
            "s": np.ascontiguousarray(s[:, nb * N_CORE:(nb + 1) * N_CORE]),
        })
    res = run_bass_kernel_spmd(
        _get_nc(), in_maps, core_ids=list(range(N_CORES)), trace=trace
    )
    out = np.empty((M_FULL, F), dtype=np.float32)
    for c in range(N_CORES):
        mb, nb = c % MSH, c // MSH
        oc = res.results[c]["o"].reshape(WM, WN, P, NT).transpose(0, 2, 1, 3)
        out[mb * M_CORE:(mb + 1) * M_CORE, nb * N_CORE:(nb + 1) * N_CORE] = \
            oc.reshape(M_CORE, N_CORE)
    return out.reshape(B, S, F), res


def kernel(x, qkernel, qscale):
    try:
        out, _ = _run(x, qkernel, qscale, trace=False)
    except Exception:
        # One retry for transient device-side failures.
        out, _ = _run(x, qkernel, qscale, trace=False)
    return out


def kernel_traced(x, qkernel, qscale):
    out, res = _run(x, qkernel, qscale, trace=True)
    return out, res


# revision 7
# speedup vs baseline: 1.2765x; 1.2765x over previous
"""DenseGeneralAqt inference kernel for Trainium2 (8 NeuronCores).

out = (x @ dequant_int8(qkernel)) * qscale,  x:(2,2048,1024) f32,
qkernel:(1024,4096) int8, qscale:(1,4096) f32 -> out:(2,2048,4096) f32.

Strategy: 2D sharding — 4-way over the flattened token axis (M) x 2-way
over features (N); per core a [1024,1024]x[1024,2048] fp16 GEMM whose
PE-streaming floor (256 matmuls of 512 cycles) dominates, so everything
else is arranged to hide under it. Host marshalling pre-packs all DRAM
operands in on-chip consumption order with 2KB-per-partition contiguous
runs: xT pair-major fp16 [128kp, pair, kt, 256m], weights kt-major int8,
output tile-major f32 [32, 128, 512]. The PE is heated with N=512 dummy
matmuls (full duty cycle -> the HAM clock gate releases 1.2->2.4 GHz
right as the first inputs land); weight k-tiles are dequantized
int8->fp16 on the vector engine just ahead of consumption; the fp16
per-channel scale is fused into the PSUM->SBUF drain. Sweeps run
k-outer across all 8 PSUM banks; the final sweep runs n-outer and its
last chain is split into two 256-wide halves so the closing drain+store
is short. Output stores alternate between the Scalar and Sync rings.
"""

import numpy as np

P = 128
B, S, D, F = 2, 2048, 1024, 4096
N_CORES = 8
MSH, NSH = 4, 2                   # shard grid: 4 m-blocks x 2 n-blocks
M_FULL = B * S                    # 4096 rows
M_CORE = M_FULL // MSH            # 1024 rows per core
N_CORE = F // NSH                 # 2048 cols per core
NT = 512                          # n-tile (one PSUM bank of f32)
WM, WK, WN = M_CORE // P, D // P, N_CORE // NT   # 8, 8, 4
NPAIR = WM // 2                   # 4 m-pair sweeps
MP = M_CORE // NPAIR              # 256 m per pair
NWARM = 8

_CACHE: dict = {}


def _build():
    import concourse.tile as tile
    from concourse import bacc, mybir

    nc = bacc.Bacc("TRN2", target_bir_lowering=False, debug=False)

    xt_dram = nc.dram_tensor("xt", [P, NPAIR, WK, MP], mybir.dt.float16, kind="ExternalInput")
    w_dram = nc.dram_tensor("w", [P, WK, N_CORE], mybir.dt.int8, kind="ExternalInput")
    s_dram = nc.dram_tensor("s", [1, N_CORE], mybir.dt.float16, kind="ExternalInput")
    o_dram = nc.dram_tensor("o", [WM * WN, P, NT], mybir.dt.float32, kind="ExternalOutput")

    with tile.TileContext(nc) as tc:
        with (
            tc.tile_pool(name="wi", bufs=1) as wip,
            tc.tile_pool(name="w", bufs=1) as wp,
            tc.tile_pool(name="qs", bufs=1) as qp,
            tc.tile_pool(name="xh", bufs=1) as xhp,
            tc.tile_pool(name="o", bufs=10) as op,
            tc.tile_pool(name="ps", bufs=8, space="PSUM") as pp,
        ):
            # Weights (int8, kt-major, 2KB-per-partition contiguous runs):
            # one DMA per k-tile, first thing on the GpSimd ring so k0
            # lands earliest and k-tiles arrive in consumption order.
            w_i8 = [
                wip.tile([P, N_CORE], mybir.dt.int8, name=f"wi{kt}", tag=f"wi{kt}")
                for kt in range(WK)
            ]
            for kt in range(WK):
                nc.gpsimd.dma_start(w_i8[kt][:], w_dram[:, kt, :])

            # PE warm-up: N=512 dummy matmuls run the array at full duty
            # (LDWEIGHTS hides under streaming), so the HAM clock gate
            # reliably releases ~3.4us after the first one issues — right
            # when the first real inputs land. Memset on the vector
            # engine, whose first real work (dequant) is DMA-gated anyway.
            warm = wp.tile([P, NT], mybir.dt.float16, name="warm", tag="warm")
            nc.vector.memset(warm[:], 0)
            warm_ps = pp.tile([P, NT], mybir.dt.float32, name="warm_ps", tag="ps")
            for _ in range(NWARM):
                nc.tensor.matmul(warm_ps[:], warm[:, 0:P], warm[:])

            # xT shard, pair-major, in half-pair chunks (2KB-per-partition
            # runs) on the Sync ring, in consumption order.
            xh = xhp.tile([P, NPAIR, WK, MP], mybir.dt.float16, name="xh", tag="xh")
            for pr in range(NPAIR):
                for hk in range(2):
                    nc.sync.dma_start(
                        xh[:, pr, 4 * hk:4 * hk + 4, :],
                        xt_dram[:, pr, 4 * hk:4 * hk + 4, :],
                    )

            # Dequant int8 -> fp16 on the vector engine, in k order; k0
            # in halves so the first matmuls' columns are ready earliest.
            w_sb = [
                wp.tile([P, N_CORE], mybir.dt.float16, name=f"w{kt}", tag=f"w{kt}")
                for kt in range(WK)
            ]
            wh = N_CORE // 2
            nc.vector.tensor_copy(w_sb[0][:, 0:wh], w_i8[0][:, 0:wh])
            nc.vector.tensor_copy(w_sb[0][:, wh:N_CORE], w_i8[0][:, wh:N_CORE])
            cv = [nc.vector.tensor_copy(w_sb[kt][:], w_i8[kt][:]) for kt in range(1, WK)]

            # Per-channel scale, fp16, replicated across partitions by a
            # DRE-broadcast DMA; deferred so its bytes don't starve the
            # critical early loads, lands long before the first drain.
            qs = qp.tile([P, N_CORE], mybir.dt.float16)
            qs_dma = nc.scalar.dma_start(qs[:], s_dram[0:1, :].to_broadcast((P, N_CORE)))
            tile.add_dep_helper(qs_dma.ins, cv[1].ins, reason="defer qs broadcast")

            st_eng = [nc.scalar, nc.sync]

            def drain(pi, mh, nt, ps_ap, cols):
                mi = pi * 2 + mh
                j = mi * WN + nt
                ot = op.tile([P, NT], mybir.dt.float32, name=f"o{j}_{cols.start}", tag="o")
                nc.vector.tensor_mul(
                    ot[:, cols], ps_ap, qs[:, nt * NT + cols.start:nt * NT + cols.stop]
                )
                st_eng[(j + cols.start // NT) % 2].dma_start(
                    o_dram[j, :, cols], ot[:, cols]
                )

            def mm(ps_ap, pi, kt, mh, nslice, first, last):
                nc.tensor.matmul(
                    ps_ap,
                    xh[:, pi, kt, mh * P:(mh + 1) * P],
                    w_sb[kt][:, nslice],
                    start=first,
                    stop=last,
                )

            full = slice(0, NT)
            combos = [(mh, nt) for mh in (0, 1) for nt in range(WN)]
            for pi in range(NPAIR):
                if pi < NPAIR - 1:
                    # k-outer: consume each weight k-tile across all 8
                    # PSUM banks as soon as it is dequantized.
                    ps = {
                        c: pp.tile([P, NT], mybir.dt.float32, name=f"ps{pi}_{c[0]}_{c[1]}", tag="ps")
                        for c in combos
                    }
                    for kt in range(WK):
                        for (mh, nt) in combos:
                            mm(ps[(mh, nt)][:], pi, kt, mh,
                               slice(nt * NT, (nt + 1) * NT), kt == 0, kt == WK - 1)
                    for (mh, nt) in combos:
                        drain(pi, mh, nt, ps[(mh, nt)][:], full)
                else:
                    # Last sweep: n-outer so each bank finishes early and
                    # drains/stores overlap the remaining matmuls; the
                    # final chain is split into two 256-wide halves so
                    # the closing drain+store is half-length.
                    for (mh, nt) in combos[:-1]:
                        ps_t = pp.tile([P, NT], mybir.dt.float32, name=f"ps{pi}_{mh}_{nt}", tag="ps")
                        for kt in range(WK):
                            mm(ps_t[:], pi, kt, mh,
                               slice(nt * NT, (nt + 1) * NT), kt == 0, kt == WK - 1)
                        drain(pi, mh, nt, ps_t[:], full)
                    mh, nt = combos[-1]
                    for half in range(2):
                        cols = slice(half * (NT // 2), (half + 1) * (NT // 2))
                        ps_t = pp.tile([P, NT], mybir.dt.float32, name=f"ps{pi}_{mh}_{nt}_{half}", tag="ps")
                        for kt in range(WK):
                            mm(ps_t[:, cols], pi, kt, mh,
                               slice(nt * NT + cols.start, nt * NT + cols.stop),
                               kt == 0, kt == WK - 1)
                        drain(pi, mh, nt, ps_t[:, cols], cols)

    nc.compile()
    return nc


def _get_nc():
    if "nc" not in _CACHE:
        _CACHE["nc"] = _build()
    return _CACHE["nc"]


def _run(x, qkernel, qscale, trace=False):
    from concourse.bass_utils import run_bass_kernel_spmd

    x = np.asarray(x, dtype=np.float32).reshape(M_FULL, D).astype(np.float16)
    w = np.asarray(qkernel)
    if w.dtype != np.int8:
        w = w.astype(np.int8)
    s = np.asarray(qscale, dtype=np.float32).reshape(1, F).astype(np.float16)

    in_maps = []
    for c in range(N_CORES):
        mb, nb = c % MSH, c // MSH
        xm = x[mb * M_CORE:(mb + 1) * M_CORE]                  # [1024, 1024]
        # [kp, pair, kt, m']  <-  xm[pr*256+m', kt*128+kp]
        xt = np.ascontiguousarray(
            xm.reshape(NPAIR, MP, WK, P).transpose(3, 0, 2, 1)
        )
        wn = w[:, nb * N_CORE:(nb + 1) * N_CORE]               # [1024, 2048]
        wk = np.ascontiguousarray(wn.reshape(WK, P, N_CORE).transpose(1, 0, 2))
        in_maps.append({
            "xt": xt,
            "w": wk,
            "s": np.ascontiguousarray(s[:, nb * N_CORE:(nb + 1) * N_CORE]),
        })
    res = run_bass_kernel_spmd(
        _get_nc(), in_maps, core_ids=list(range(N_CORES)), trace=trace
    )
    out = np.empty((M_FULL, F), dtype=np.float32)
    for c in range(N_CORES):
        mb, nb = c % MSH, c // MSH
        oc = res.results[c]["o"].reshape(WM, WN, P, NT).transpose(0, 2, 1, 3)
        out[mb * M_CORE:(mb + 1) * M_CORE, nb * N_CORE:(nb + 1) * N_CORE] = \
            oc.reshape(M_CORE, N_CORE)
    return out.reshape(B, S, F), res


def kernel(x, qkernel, qscale):
    try:
        out, _ = _run(x, qkernel, qscale, trace=False)
    except Exception:
        # One retry for transient device-side failures.
        out, _ = _run(x, qkernel, qscale, trace=False)
    return out


def kernel_traced(x, qkernel, qscale):
    out, res = _run(x, qkernel, qscale, trace=True)
    return out, res


# revision 9
# speedup vs baseline: 1.2808x; 1.0033x over previous
"""DenseGeneralAqt inference kernel for Trainium2 (8 NeuronCores).

out = (x @ dequant_int8(qkernel)) * qscale,  x:(2,2048,1024) f32,
qkernel:(1024,4096) int8, qscale:(1,4096) f32 -> out:(2,2048,4096) f32.

Strategy: 2D sharding — 4-way over the flattened token axis (M) x 2-way
over features (N); per core a [1024,1024]x[1024,2048] fp16 GEMM whose
PE-streaming floor (256 matmuls of 512 cycles) dominates, so everything
else is arranged to hide under it. Host marshalling pre-packs all DRAM
operands in on-chip consumption order with 2KB-per-partition contiguous
runs: xT pair-major fp16 [128kp, pair, kt, 256m], weights kt-major int8,
output tile-major f32 [32, 128, 512]. The PE is heated with N=512 dummy
matmuls (full duty cycle -> the HAM clock gate releases 1.2->2.4 GHz
right as the first inputs land); weight k-tiles are dequantized
int8->fp16 on the vector engine just ahead of consumption; the fp16
per-channel scale is fused into the PSUM->SBUF drain. Sweeps run
k-outer across all 8 PSUM banks; the final sweep runs n-outer and its
last chain is split into two 256-wide halves so the closing drain+store
is short. Output stores alternate between the Scalar and Sync rings.
"""

import numpy as np

P = 128
B, S, D, F = 2, 2048, 1024, 4096
N_CORES = 8
MSH, NSH = 4, 2                   # shard grid: 4 m-blocks x 2 n-blocks
M_FULL = B * S                    # 4096 rows
M_CORE = M_FULL // MSH            # 1024 rows per core
N_CORE = F // NSH                 # 2048 cols per core
NT = 512                          # n-tile (one PSUM bank of f32)
WM, WK, WN = M_CORE // P, D // P, N_CORE // NT   # 8, 8, 4
NPAIR = WM // 2                   # 4 m-pair sweeps
MP = M_CORE // NPAIR              # 256 m per pair
NWARM = 8

_CACHE: dict = {}


def _build():
    import concourse.tile as tile
    from concourse import bacc, mybir

    nc = bacc.Bacc("TRN2", target_bir_lowering=False, debug=False)

    xt_dram = nc.dram_tensor("xt", [P, NPAIR, WK, MP], mybir.dt.float16, kind="ExternalInput")
    w_dram = nc.dram_tensor("w", [P, WK, N_CORE], mybir.dt.int8, kind="ExternalInput")
    s_dram = nc.dram_tensor("s", [1, N_CORE], mybir.dt.float16, kind="ExternalInput")
    o_dram = nc.dram_tensor("o", [WM * WN, P, NT], mybir.dt.float32, kind="ExternalOutput")

    with tile.TileContext(nc) as tc:
        with (
            tc.tile_pool(name="wi", bufs=1) as wip,
            tc.tile_pool(name="w", bufs=1) as wp,
            tc.tile_pool(name="qs", bufs=1) as qp,
            tc.tile_pool(name="xh", bufs=1) as xhp,
            tc.tile_pool(name="o", bufs=10) as op,
            tc.tile_pool(name="ps", bufs=8, space="PSUM") as pp,
        ):
            # Weights (int8, kt-major, 2KB-per-partition contiguous runs):
            # one DMA per k-tile on the Sync ring (the earliest issuer
            # post-barrier) so k0 lands first and k-tiles arrive in
            # consumption order.
            w_i8 = [
                wip.tile([P, N_CORE], mybir.dt.int8, name=f"wi{kt}", tag=f"wi{kt}")
                for kt in range(WK)
            ]
            for kt in range(WK):
                nc.sync.dma_start(w_i8[kt][:], w_dram[:, kt, :])

            # PE warm-up: N=512 dummy matmuls run the array at full duty
            # (LDWEIGHTS hides under streaming), so the HAM clock gate
            # reliably releases ~3.4us after the first one issues — right
            # when the first real inputs land. Memset on the vector
            # engine, whose first real work (dequant) is DMA-gated anyway.
            warm = wp.tile([P, NT], mybir.dt.float16, name="warm", tag="warm")
            nc.vector.memset(warm[:], 0)
            warm_ps = pp.tile([P, NT], mybir.dt.float32, name="warm_ps", tag="ps")
            for _ in range(NWARM):
                nc.tensor.matmul(warm_ps[:], warm[:, 0:P], warm[:])

            # xT shard, pair-major, in half-pair chunks (2KB-per-partition
            # runs) on the GpSimd ring. Only pair 0 is needed at stream
            # start; pairs 1-3 (1.5 MB) are deferred behind early dequant
            # casts so their bytes don't sit ahead of the critical weight
            # k-tiles in the shared DMA queues.
            xh = xhp.tile([P, NPAIR, WK, MP], mybir.dt.float16, name="xh", tag="xh")
            xd = []
            for pr in range(NPAIR):
                for hk in range(2):
                    xd.append(nc.gpsimd.dma_start(
                        xh[:, pr, 4 * hk:4 * hk + 4, :],
                        xt_dram[:, pr, 4 * hk:4 * hk + 4, :],
                    ))

            # Dequant int8 -> fp16 on the vector engine, in k order; k0
            # in halves so the first matmuls' columns are ready earliest.
            w_sb = [
                wp.tile([P, N_CORE], mybir.dt.float16, name=f"w{kt}", tag=f"w{kt}")
                for kt in range(WK)
            ]
            wh = N_CORE // 2
            nc.vector.tensor_copy(w_sb[0][:, 0:wh], w_i8[0][:, 0:wh])
            nc.vector.tensor_copy(w_sb[0][:, wh:N_CORE], w_i8[0][:, wh:N_CORE])
            cv = [nc.vector.tensor_copy(w_sb[kt][:], w_i8[kt][:]) for kt in range(1, WK)]
            tile.add_dep_helper(xd[2].ins, cv[0].ins, reason="defer xh p1")
            tile.add_dep_helper(xd[4].ins, cv[2].ins, reason="defer xh p2")
            tile.add_dep_helper(xd[6].ins, cv[4].ins, reason="defer xh p3")

            # Per-channel scale, fp16, replicated across partitions by a
            # DRE-broadcast DMA; deferred so its bytes don't starve the
            # critical early loads, lands long before the first drain.
            qs = qp.tile([P, N_CORE], mybir.dt.float16)
            qs_dma = nc.scalar.dma_start(qs[:], s_dram[0:1, :].to_broadcast((P, N_CORE)))
            tile.add_dep_helper(qs_dma.ins, cv[1].ins, reason="defer qs broadcast")

            st_eng = [nc.scalar, nc.sync]

            def drain(pi, mh, nt, ps_ap, cols):
                mi = pi * 2 + mh
                j = mi * WN + nt
                ot = op.tile([P, NT], mybir.dt.float32, name=f"o{j}_{cols.start}", tag="o")
                nc.vector.tensor_mul(
                    ot[:, cols], ps_ap, qs[:, nt * NT + cols.start:nt * NT + cols.stop]
                )
                st_eng[(j + cols.start // NT) % 2].dma_start(
                    o_dram[j, :, cols], ot[:, cols]
                )

            def mm(ps_ap, pi, kt, mh, nslice, first, last):
                nc.tensor.matmul(
                    ps_ap,
                    xh[:, pi, kt, mh * P:(mh + 1) * P],
                    w_sb[kt][:, nslice],
                    start=first,
                    stop=last,
                )

            full = slice(0, NT)
            combos = [(mh, nt) for mh in (0, 1) for nt in range(WN)]
            for pi in range(NPAIR):
                if pi < NPAIR - 1:
                    # k-outer: consume each weight k-tile across all 8
                    # PSUM banks as soon as it is dequantized.
                    ps = {
                        c: pp.tile([P, NT], mybir.dt.float32, name=f"ps{pi}_{c[0]}_{c[1]}", tag="ps")
                        for c in combos
                    }
                    for kt in range(WK):
                        for (mh, nt) in combos:
                            mm(ps[(mh, nt)][:], pi, kt, mh,
                               slice(nt * NT, (nt + 1) * NT), kt == 0, kt == WK - 1)
                    for (mh, nt) in combos:
                        drain(pi, mh, nt, ps[(mh, nt)][:], full)
                else:
                    # Last sweep: n-outer so each bank finishes early and
                    # drains/stores overlap the remaining matmuls; the
                    # final chain is split into two 256-wide halves so
                    # the closing drain+store is half-length.
                    for (mh, nt) in combos[:-1]:
                        ps_t = pp.tile([P, NT], mybir.dt.float32, name=f"ps{pi}_{mh}_{nt}", tag="ps")
                        for kt in range(WK):
                            mm(ps_t[:], pi, kt, mh,
                               slice(nt * NT, (nt + 1) * NT), kt == 0, kt == WK - 1)
                        drain(pi, mh, nt, ps_t[:], full)
                    mh, nt = combos[-1]
                    for half in range(2):
                        cols = slice(half * (NT // 2), (half + 1) * (NT // 2))
                        ps_t = pp.tile([P, NT], mybir.dt.float32, name=f"ps{pi}_{mh}_{nt}_{half}", tag="ps")
                        for kt in range(WK):
                            mm(ps_t[:, cols], pi, kt, mh,
                               slice(nt * NT + cols.start, nt * NT + cols.stop),
                               kt == 0, kt == WK - 1)
                        drain(pi, mh, nt, ps_t[:, cols], cols)

    nc.compile()
    return nc


def _get_nc():
    if "nc" not in _CACHE:
        _CACHE["nc"] = _build()
    return _CACHE["nc"]


def _run(x, qkernel, qscale, trace=False):
    from concourse.bass_utils import run_bass_kernel_spmd

    x = np.asarray(x, dtype=np.float32).reshape(M_FULL, D).astype(np.float16)
    w = np.asarray(qkernel)
    if w.dtype != np.int8:
        w = w.astype(np.int8)
    s = np.asarray(qscale, dtype=np.float32).reshape(1, F).astype(np.float16)

    in_maps = []
    for c in range(N_CORES):
        mb, nb = c % MSH, c // MSH
        xm = x[mb * M_CORE:(mb + 1) * M_CORE]                  # [1024, 1024]
        # [kp, pair, kt, m']  <-  xm[pr*256+m', kt*128+kp]
        xt = np.ascontiguousarray(
            xm.reshape(NPAIR, MP, WK, P).transpose(3, 0, 2, 1)
        )
        wn = w[:, nb * N_CORE:(nb + 1) * N_CORE]               # [1024, 2048]
        wk = np.ascontiguousarray(wn.reshape(WK, P, N_CORE).transpose(1, 0, 2))
        in_maps.append({
            "xt": xt,
            "w": wk,
            "s": np.ascontiguousarray(s[:, nb * N_CORE:(nb + 1) * N_CORE]),
        })
    res = run_bass_kernel_spmd(
        _get_nc(), in_maps, core_ids=list(range(N_CORES)), trace=trace
    )
    out = np.empty((M_FULL, F), dtype=np.float32)
    for c in range(N_CORES):
        mb, nb = c % MSH, c // MSH
        oc = res.results[c]["o"].reshape(WM, WN, P, NT).transpose(0, 2, 1, 3)
        out[mb * M_CORE:(mb + 1) * M_CORE, nb * N_CORE:(nb + 1) * N_CORE] = \
            oc.reshape(M_CORE, N_CORE)
    return out.reshape(B, S, F), res


def kernel(x, qkernel, qscale):
    try:
        out, _ = _run(x, qkernel, qscale, trace=False)
    except Exception:
        # One retry for transient device-side failures.
        out, _ = _run(x, qkernel, qscale, trace=False)
    return out


def kernel_traced(x, qkernel, qscale):
    out, res = _run(x, qkernel, qscale, trace=True)
    return out, res
